# revision 3
# baseline (speedup 1.0000x reference)
"""GNN message-passing kernel for Trainium2 (8 NeuronCores).

Reference computation:
    out[b,i,f] = X[b,0,i,i,f] + sum_{k=1..3} sum_j A[b,i,j] * X[b,k,i,j,f]

Sharding: 8 cores = (batch b in 0..3) x (i-half h in 0..1); each core owns
a (b, 128-row i-slab). Hop 0 only contributes its diagonal, so only
X[b,1:4] (3/4 of X) plus the hop-0 diagonal rows are ever sent to the
device: ~24.7 MB per core.

Per-core device kernel: for each output row i, out_row[f] =
A[i,:] @ Xs[i,:,:] is computed on the TensorEngine as 6 accumulating
matmuls (2 j-chunks of 128 x 3 hops) with stationary = one column of
A^T, moving = the (128j x 64f) X tile, accumulated in PSUM. The hop-0
diagonal is added during PSUM eviction on the VectorEngine.
"""

import sys

if "/opt/trn_rl_repo" not in sys.path:
    sys.path.insert(0, "/opt/trn_rl_repo")

import numpy as np

import concourse.bacc as bacc
import concourse.bass as bass
import concourse.mybir as mybir
from concourse.bass_utils import run_bass_kernel_spmd
from concourse.tile import TileContext

BATCH, KP1, N, F = 4, 4, 256, 64
NH = N // 2          # 128 rows of output per core
G = 16               # i-rows per group (1 MB X DMA per hop)
NG = NH // G         # groups per core
FP32 = mybir.dt.float32

_CACHE = {}


def _build_nc():
    if "nc" in _CACHE:
        return _CACHE["nc"]
    nc = bacc.Bacc("TRN2", target_bir_lowering=False, debug=False, num_devices=8)
    xk = nc.dram_tensor("xk", [3, NH, N, F], FP32, kind="ExternalInput").ap()
    at = nc.dram_tensor("at", [2, 128, NH], FP32, kind="ExternalInput").ap()
    dg = nc.dram_tensor("dg", [NG, G * F], FP32, kind="ExternalInput").ap()
    out = nc.dram_tensor("out", [NG, G * F], FP32, kind="ExternalOutput").ap()

    with TileContext(nc) as tc:
        with (
            tc.tile_pool(name="const", bufs=1) as cpool,
            tc.tile_pool(name="xpool", bufs=2) as xpool,
            tc.tile_pool(name="dpool", bufs=2) as dpool,
            tc.tile_pool(name="opool", bufs=2) as opool,
            tc.tile_pool(name="ppool", bufs=2, space="PSUM") as ppool,
        ):
            # A^T in SBUF: partition = j (within chunk), free = jc*128 + i
            at_sb = cpool.tile([128, 2 * NH], FP32)
            nc.sync.dma_start(out=at_sb[:, 0:NH], in_=at[0])
            nc.sync.dma_start(out=at_sb[:, NH : 2 * NH], in_=at[1])

            for g in range(NG):
                xts = []
                for k in range(3):
                    xt = xpool.tile(
                        [128, G * 2 * F], FP32, name=f"xt{k}", tag=f"xt{k}"
                    )
                    # src[p, (ii*2+jc)*F + f] = xk[k, g*G+ii, jc*128+p, f]
                    # elem strides in xk: k: NH*N*F, i: N*F, j: F, f: 1
                    # (ii, jc) merge into one dim of 2*G with step 128*F
                    src = bass.AP(
                        xk.tensor,
                        k * NH * N * F + g * G * N * F,
                        [[F, 128], [128 * F, 2 * G], [1, F]],
                    )
                    nc.sync.dma_start(out=xt[:, :], in_=src)
                    xts.append(xt)

                dgt = dpool.tile([1, G * F], FP32, name="dgt")
                nc.sync.dma_start(out=dgt[:1, :], in_=dg[g : g + 1, :])

                pt = ppool.tile([1, G * F], FP32, name="pt")
                for ii in range(G):
                    il = g * G + ii
                    for jc in range(2):
                        lw = at_sb[:, jc * NH + il : jc * NH + il + 1]
                        for k in range(3):
                            nc.tensor.matmul(
                                pt[0:1, ii * F : (ii + 1) * F],
                                lw,
                                xts[k][:, ii * 2 * F + jc * F : ii * 2 * F + (jc + 1) * F],
                                start=(jc == 0 and k == 0),
                                stop=(jc == 1 and k == 2),
                            )

                ot = opool.tile([1, G * F], FP32, name="ot")
                nc.vector.tensor_add(ot[:1, :], pt[0:1, :], dgt[:1, :])
                nc.sync.dma_start(out=out[g : g + 1, :], in_=ot[:1, :])

    nc.compile()
    _CACHE["nc"] = nc
    return nc


def _make_in_maps(A, X):
    idx = np.arange(NH)
    in_maps = []
    for c in range(8):
        b, h = c // 2, c % 2
        lo = h * NH
        xk = np.ascontiguousarray(X[b, 1:4, lo : lo + NH])
        at = np.ascontiguousarray(
            A[b, lo : lo + NH, :].T.reshape(2, 128, NH)
        )
        dgv = np.ascontiguousarray(
            X[b, 0, lo + idx, lo + idx, :].reshape(NG, G * F)
        )
        in_maps.append({"xk": xk, "at": at, "dg": dgv})
    return in_maps


def run(A, X, trace=False, **kw):
    nc = _build_nc()
    in_maps = _make_in_maps(A, X)
    res = run_bass_kernel_spmd(
        nc, in_maps, core_ids=list(range(8)), trace=trace, **kw
    )
    out = np.empty((BATCH, N, F), dtype=np.float32)
    for c in range(8):
        b, h = c // 2, c % 2
        out[b, h * NH : (h + 1) * NH] = res.results[c]["out"].reshape(NH, F)
    return out, res


def kernel(A, X):
    A = np.asarray(A, dtype=np.float32)
    X = np.asarray(X, dtype=np.float32)
    out, _ = run(A, X, trace=False)
    return out


# revision 5
# speedup vs baseline: 1.4530x; 1.4530x over previous
"""GNN message-passing kernel for Trainium2 (8 NeuronCores).

Reference computation:
    out[b,i,f] = X[b,0,i,i,f] + sum_{k=1..3} sum_j A[b,i,j] * X[b,k,i,j,f]

Sharding: 8 cores = (batch b in 0..3) x (i-half h in 0..1); each core owns
a (b, 128-row i-slab). Hop 0 only contributes its diagonal, so only
X[b,1:4] (3/4 of X) plus the hop-0 diagonal rows are ever sent to the
device: ~25 MB per core.

Per-core device kernel (v2 — DVE formulation, no transpose):
  - X slabs are DMA'd in their NATURAL layout: partition = i (128 rows),
    free = (j, f) flattened. Each partition's data is one fully
    contiguous 16 KB run per (hop, j-chunk) -> near-peak HBM bandwidth.
  - The 3-hop sum is folded into the load itself with SWDGE
    accumulate-DMA (CCE inline add): hops 2,3 accumulate onto hop 1's
    tile. SBUF then holds Xs = sum_k X[k] directly.
  - out[i,f] = sum_j A[i,j] * Xs[i,j,f] is computed on the VectorEngine:
    a broadcast-AP multiply (A[i,j] broadcast over f via a 0-step AP
    dim) followed by tensor_reduce over j, per j-chunk, plus the hop-0
    diagonal added at the end.
"""

import sys

if "/opt/trn_rl_repo" not in sys.path:
    sys.path.insert(0, "/opt/trn_rl_repo")

import numpy as np

import concourse.bacc as bacc
import concourse.bass as bass
import concourse.mybir as mybir
from concourse.bass_utils import run_bass_kernel_spmd
from concourse.tile import TileContext

BATCH, KP1, N, F = 4, 4, 256, 64
NH = N // 2          # 128 rows of output per core (partition dim)
CH = 4               # j-chunks
CJ = N // CH         # j per chunk
FP32 = mybir.dt.float32
USE_ACCUM_DMA = False

_CACHE = {}


def _build_nc():
    if "nc" in _CACHE:
        return _CACHE["nc"]
    nc = bacc.Bacc("TRN2", target_bir_lowering=False, debug=False, num_devices=8)
    xk = nc.dram_tensor("xk", [3, NH, N, F], FP32, kind="ExternalInput").ap()
    a = nc.dram_tensor("a", [NH, N], FP32, kind="ExternalInput").ap()
    d = nc.dram_tensor("d", [NH, F], FP32, kind="ExternalInput").ap()
    out = nc.dram_tensor("out", [NH, F], FP32, kind="ExternalOutput").ap()

    with TileContext(nc) as tc:
        with (
            tc.tile_pool(name="const", bufs=1) as cpool,
            tc.tile_pool(name="xs", bufs=3) as xpool,
            tc.tile_pool(name="pr", bufs=2) as prpool,
            tc.tile_pool(name="sm", bufs=2) as smpool,
            tc.tile_pool(name="ac", bufs=1) as acpool,
        ):
            a_sb = cpool.tile([128, N], FP32)
            nc.sync.dma_start(out=a_sb[:, :], in_=a[:, :])
            d_sb = cpool.tile([128, F], FP32)
            nc.sync.dma_start(out=d_sb[:, :], in_=d[:, :])

            acc = acpool.tile([128, F], FP32)

            for c in range(CH):
                xs = xpool.tile([128, CJ * F], FP32, name="xs", tag="xs")
                xs_step = xs.ap[0][0]
                if USE_ACCUM_DMA:
                    for k in range(3):
                        src = bass.AP(
                            xk.tensor,
                            k * NH * N * F + c * CJ * F,
                            [[N * F, 128], [1, CJ * F]],
                        )
                        nc.gpsimd.dma_start(
                            out=xs[:, :],
                            in_=src,
                            accum_op=(
                                mybir.AluOpType.add
                                if k > 0
                                else mybir.AluOpType.bypass
                            ),
                        )
                else:
                    xts = [xs]
                    for k in (1, 2):
                        xt = xpool.tile(
                            [128, CJ * F], FP32, name=f"xt{k}", tag=f"xt{k}"
                        )
                        xts.append(xt)
                    for k in range(3):
                        src = bass.AP(
                            xk.tensor,
                            k * NH * N * F + c * CJ * F,
                            [[N * F, 128], [1, CJ * F]],
                        )
                        nc.sync.dma_start(out=xts[k][:, :], in_=src)
                    nc.vector.tensor_add(xts[1][:, :], xts[1][:, :], xts[2][:, :])
                    nc.vector.tensor_add(xs[:, :], xts[0][:, :], xts[1][:, :])

                # prod[i, j*F+f] = xs[i, j*F+f] * a_sb[i, c*CJ+j]
                prod = prpool.tile([128, CJ * F], FP32, name="prod", tag="prod")
                pr_step = prod.ap[0][0]
                a_step = a_sb.ap[0][0]
                in0 = bass.AP(xs.tensor, 0, [[xs_step, 128], [F, CJ], [1, F]])
                in1 = bass.AP(
                    a_sb.tensor, c * CJ, [[a_step, 128], [1, CJ], [0, F]]
                )
                po = bass.AP(prod.tensor, 0, [[pr_step, 128], [F, CJ], [1, F]])
                nc.vector.tensor_mul(po, in0, in1)

                # partial[i, f] = sum_j prod[i, j*F+f]  (reduce innermost=j)
                partial = smpool.tile([128, F], FP32, name="partial", tag="partial")
                pin = bass.AP(prod.tensor, 0, [[pr_step, 128], [1, F], [F, CJ]])
                nc.vector.reduce_sum(
                    partial[:, :], pin, axis=mybir.AxisListType.X
                )

                if c == 0:
                    nc.vector.tensor_add(acc[:, :], d_sb[:, :], partial[:, :])
                else:
                    nc.vector.tensor_add(acc[:, :], acc[:, :], partial[:, :])

            nc.sync.dma_start(out=out[:, :], in_=acc[:, :])

    nc.compile()
    _CACHE["nc"] = nc
    return nc


def _make_in_maps(A, X):
    idx = np.arange(NH)
    in_maps = []
    for c in range(8):
        b, h = c // 2, c % 2
        lo = h * NH
        xk = np.ascontiguousarray(X[b, 1:4, lo : lo + NH])
        av = np.ascontiguousarray(A[b, lo : lo + NH, :])
        dv = np.ascontiguousarray(X[b, 0, lo + idx, lo + idx, :])
        in_maps.append({"xk": xk, "a": av, "d": dv})
    return in_maps


def run(A, X, trace=False, **kw):
    nc = _build_nc()
    in_maps = _make_in_maps(A, X)
    res = run_bass_kernel_spmd(
        nc, in_maps, core_ids=list(range(8)), trace=trace, **kw
    )
    out = np.empty((BATCH, N, F), dtype=np.float32)
    for c in range(8):
        b, h = c // 2, c % 2
        out[b, h * NH : (h + 1) * NH] = res.results[c]["out"]
    return out, res


def kernel(A, X):
    A = np.asarray(A, dtype=np.float32)
    X = np.asarray(X, dtype=np.float32)
    out, _ = run(A, X, trace=False)
    return out
